# revision 12
# baseline (speedup 1.0000x reference)
"""BiGRU+CRF NER kernel for 8 TRN2 NeuronCores.

Sharding: data-parallel over batch (B=32 -> 4 per core). Each core runs:
  embedding gather (indirect DMA) -> x transpose (PE) -> xp = x@Wih.T (PE)
  -> fwd+bwd GRU scans (interleaved, transposed state layout [H=128, B])
  -> emissions -> Viterbi decode via max-marginals (fwd+bwd max-plus scans,
  no backtrace) -> CRF log-likelihood via exp-domain matmul chain.
Host only shards/permutes inputs and concatenates outputs.
"""
import os
import sys

sys.path.insert(0, '/opt/trn_rl_repo')
import numpy as np
import types
try:
    import antenv.axon_hooks  # noqa: F401
except Exception:
    import antenv
    _m = types.ModuleType('antenv.axon_hooks')
    _m.get_axon_ntff_profile_hook = lambda: None
    sys.modules['antenv.axon_hooks'] = _m

import concourse.bass as bass
import concourse.bacc as bacc
import concourse.mybir as mybir
from concourse import tile
from concourse.bass_utils import run_bass_kernel_spmd

F32 = mybir.dt.float32
I32 = mybir.dt.int32
AF = mybir.ActivationFunctionType
OP = mybir.AluOpType
AX = mybir.AxisListType

VOCAB, HID, H, T, L = 21128, 768, 128, 256, 9
B = 4                  # local batch per core
NTOK = B * T           # 1024 tokens per core, token index = t*B + b
NCORES = 8
RESC = 8               # CRF rescale interval (keep Ln inputs in spline range)


def build_nc():
    nc = bacc.Bacc(None, target_bir_lowering=False)
    ids_d = nc.declare_dram_parameter("ids", [128, NTOK // 128], I32, isOutput=False)
    lab_d = nc.declare_dram_parameter("lab", [1, NTOK], F32, isOutput=False)
    emb_d = nc.declare_dram_parameter("emb", [VOCAB, HID], F32, isOutput=False)
    wih_d = {d: nc.declare_dram_parameter(f"wihT_{d}", [HID, 3 * H], F32, isOutput=False) for d in "fb"}
    whh_d = {d: nc.declare_dram_parameter(f"whhT_{d}", [H, 3 * H], F32, isOutput=False) for d in "fb"}
    bih_d = {d: nc.declare_dram_parameter(f"bih_{d}", [3 * H], F32, isOutput=False) for d in "fb"}
    bhh_d = {d: nc.declare_dram_parameter(f"bhh_{d}", [3 * H], F32, isOutput=False) for d in "fb"}
    wlin_d = nc.declare_dram_parameter("wlinT", [2 * H, L], F32, isOutput=False)
    blin_d = nc.declare_dram_parameter("blin", [L], F32, isOutput=False)
    trans_d = nc.declare_dram_parameter("trans", [L, L], F32, isOutput=False)
    start_d = nc.declare_dram_parameter("start", [L], F32, isOutput=False)
    end_d = nc.declare_dram_parameter("end", [L], F32, isOutput=False)
    odec_d = nc.declare_dram_parameter("out_dec", [B, T], I32, isOutput=True)
    ostat_d = nc.declare_dram_parameter("out_stat", [1, 4 * B], F32, isOutput=True)
    e_dram = nc.dram_tensor("e_scratch", [NTOK, L], F32)

    JT = NTOK // 128  # 8 token tiles

    with tile.TileContext(nc) as tc:
        with tc.tile_pool(name="persist", bufs=1) as P:
            # ---- persistent tiles ----
            ident = P.tile([128, 128], F32, tag="ident")
            ones_row = P.tile([1, 128], F32, tag="ones_row")
            ones9 = P.tile([L, 1], F32, tag="ones9")
            iota9f = P.tile([B, L], F32, tag="iota9f")
            iota9v = P.tile([L, 1], F32, tag="iota9v")
            whh_sb = {d: P.tile([H, 3 * H], F32, tag=f"whh_{d}", name=f"whh_{d}") for d in "fb"}
            wlin_sb = P.tile([H, 2, L], F32, tag="wlin_sb")
            blin_sb = P.tile([L, 1], F32, tag="blin_sb")
            trans_sb = P.tile([L, L], F32, tag="trans_sb")
            trt_row = P.tile([1, L * L], F32, tag="trt_row")
            tr_row = P.tile([1, L * L], F32, tag="tr_row")
            startv = P.tile([L, 1], F32, tag="startv")
            endv = P.tile([L, 1], F32, tag="endv")
            start_row = P.tile([1, L], F32, tag="start_row")
            end_row = P.tile([1, L], F32, tag="end_row")
            xp = {d: P.tile([128, 3, NTOK], F32, tag=f"xp_{d}", name=f"xp_{d}") for d in "fb"}
            hT = {d: P.tile([128, NTOK], F32, tag=f"hT_{d}", name=f"hT_{d}") for d in "fb"}
            emisT = P.tile([L, NTOK], F32, tag="emisT")
            E_dec = P.tile([B, T, L], F32, tag="E_dec")
            A_buf = P.tile([B, T, L], F32, tag="A_buf")
            V_buf = P.tile([B, T, L], F32, tag="V_buf")
            TRT_rep = P.tile([B, L, L], F32, tag="TRT_rep")
            TR_rep = P.tile([B, L, L], F32, tag="TR_rep")
            expe = P.tile([L, NTOK], F32, tag="expe")
            exptr = P.tile([L, L], F32, tag="exptr")
            exps = P.tile([L, 1], F32, tag="exps")
            expend = P.tile([L, 1], F32, tag="expend")
            logsc = P.tile([1, B], F32, tag="logsc")
            lab_row = P.tile([1, NTOK], F32, tag="lab_row")
            YT = P.tile([L, NTOK], F32, tag="YT")

            # ================= phase A: consts + weights =================
            with tc.tile_pool(name="sbA", bufs=1) as SA, \
                 tc.tile_pool(name="psA", bufs=2, space="PSUM") as PSA, \
                 tc.tile_pool(name="psX", bufs=2, space="PSUM") as PSX, \
                 tc.tile_pool(name="cpA", bufs=4) as CPA:
                ids_sb = SA.tile([128, JT], I32, tag="ids_sb")
                nc.sync.dma_start(out=ids_sb[:, :], in_=ids_d[:, :])
                nc.sync.dma_start(out=lab_row[:, :], in_=lab_d[:, :])
                wih_sb = {d: SA.tile([128, HID // 128, 3 * H], F32, tag=f"wih_{d}", name=f"wih_{d}") for d in "fb"}
                for d in "fb":
                    nc.sync.dma_start(out=wih_sb[d][:, :, :],
                                      in_=wih_d[d][:, :].rearrange("(c p) n -> p c n", p=128))
                    nc.sync.dma_start(out=whh_sb[d][:, :], in_=whh_d[d][:, :])
                badd = {}
                for d in "fb":
                    bi = SA.tile([128, 3], F32, tag=f"bi_{d}")
                    bh = SA.tile([128, 3], F32, tag=f"bh_{d}")
                    nc.sync.dma_start(out=bi[:, :], in_=bih_d[d][:].rearrange("(g p) -> p g", p=128))
                    nc.sync.dma_start(out=bh[:, :], in_=bhh_d[d][:].rearrange("(g p) -> p g", p=128))
                    ba = SA.tile([128, 3], F32, tag=f"ba_{d}")
                    nc.vector.tensor_tensor(out=ba[:, :], in0=bi[:, :], in1=bh[:, :], op=OP.add)
                    badd[d] = ba
                nc.sync.dma_start(out=wlin_sb[:, :, :],
                                  in_=wlin_d[:, :].rearrange("(h p) n -> p h n", p=128))
                nc.sync.dma_start(out=blin_sb[:, :], in_=blin_d[:].unsqueeze(1))
                nc.sync.dma_start(out=trans_sb[:, :], in_=trans_d[:, :])
                nc.sync.dma_start(out=tr_row[0:1, :].rearrange("o (i j) -> o i j", i=L),
                                  in_=trans_d[:, :].unsqueeze(0))
                nc.sync.dma_start(out=trt_row[0:1, :].rearrange("o (j i) -> o j i", j=L),
                                  in_=trans_d[:, :].transpose([1, 0]).unsqueeze(0))
                nc.sync.dma_start(out=startv[:, :], in_=start_d[:].unsqueeze(1))
                nc.sync.dma_start(out=endv[:, :], in_=end_d[:].unsqueeze(1))
                nc.sync.dma_start(out=start_row[:, :], in_=start_d[:].unsqueeze(0))
                nc.sync.dma_start(out=end_row[:, :], in_=end_d[:].unsqueeze(0))
                nc.vector.memset(ones_row[:, :], 1.0)
                nc.vector.memset(ones9[:, :], 1.0)
                nc.vector.memset(logsc[:, :], 0.0)
                # identity matrix + iotas
                ia_i = SA.tile([128, 128], I32, tag="ia_i")
                ib_i = SA.tile([128, 1], I32, tag="ib_i")
                nc.gpsimd.iota(ia_i[:, :], pattern=[[1, 128]], channel_multiplier=0)
                nc.gpsimd.iota(ib_i[:, :], pattern=[[0, 1]], channel_multiplier=1)
                ia_f = SA.tile([128, 128], F32, tag="ia_f")
                ib_f = SA.tile([128, 1], F32, tag="ib_f")
                nc.vector.tensor_copy(ia_f[:, :], ia_i[:, :])
                nc.vector.tensor_copy(ib_f[:, :], ib_i[:, :])
                nc.vector.tensor_tensor(out=ident[:, :], in0=ia_f[:, :],
                                        in1=ib_f[:, 0:1].to_broadcast([128, 128]), op=OP.is_equal)
                i9 = SA.tile([B, L], I32, tag="i9")
                nc.gpsimd.iota(i9[:, :], pattern=[[1, L]], channel_multiplier=0)
                nc.vector.tensor_copy(iota9f[:, :], i9[:, :])
                i9v = SA.tile([L, 1], I32, tag="i9v")
                nc.gpsimd.iota(i9v[:, :], pattern=[[0, 1]], channel_multiplier=1)
                nc.vector.tensor_copy(iota9v[:, :], i9v[:, :])

                # ============ phase B: embedding gather ============
                x_sb = SA.tile([128, JT, HID], F32, tag="x_sb")
                for j in range(JT):
                    nc.gpsimd.indirect_dma_start(
                        out=x_sb[:, j, :], out_offset=None,
                        in_=emb_d[:, :],
                        in_offset=bass.IndirectOffsetOnAxis(ap=ids_sb[:, j:j + 1], axis=0))

                # ============ phase C: transpose x ============
                xT = SA.tile([128, HID // 128, NTOK], F32, tag="xT")
                k = 0
                for j in range(JT):
                    for c in range(HID // 128):
                        pst = PSA.tile([128, 128], F32)
                        nc.tensor.transpose(out=pst[:, :], in_=x_sb[:, j, c * 128:(c + 1) * 128],
                                            identity=ident[:, :])
                        dst = xT[:, c, j * 128:(j + 1) * 128]
                        if k % 2 == 0:
                            nc.vector.tensor_copy(dst, pst[:, :])
                        else:
                            nc.scalar.activation(dst, pst[:, :], AF.Copy)
                        k += 1

                # ============ phase D: xp = x @ WihT + (bih+bhh) ============
                for d in "fb":
                    for g in range(3):
                        for n in range(2):
                            psx = PSX.tile([128, 512], F32)
                            for c in range(HID // 128):
                                nc.tensor.matmul(psx[:, :],
                                                 lhsT=wih_sb[d][:, c, g * H:(g + 1) * H],
                                                 rhs=xT[:, c, n * 512:(n + 1) * 512],
                                                 start=(c == 0), stop=(c == HID // 128 - 1))
                            nc.vector.tensor_scalar(
                                out=xp[d][:, g, n * 512:(n + 1) * 512], in0=psx[:, :],
                                scalar1=badd[d][:, g:g + 1], scalar2=None, op0=OP.add)

            # ============ phase E: GRU scans (fwd & bwd interleaved) ============
            with tc.tile_pool(name="ps_rz", bufs=3, space="PSUM") as PRZ, \
                 tc.tile_pool(name="ps_n", bufs=3, space="PSUM") as PN, \
                 tc.tile_pool(name="gruw", bufs=4) as GW:
                def gru_step(d, tprev_col, t_col, first):
                    w = whh_sb[d]
                    prev = hT[d][:, tprev_col * B:(tprev_col + 1) * B] if not first else None
                    xpr = xp[d][:, 0:2, t_col * B:(t_col + 1) * B]  # [128,2,B]
                    xpn = xp[d][:, 2, t_col * B:(t_col + 1) * B]    # [128,B]
                    out_h = hT[d][:, t_col * B:(t_col + 1) * B]
                    rz = GW.tile([128, 2 * B], F32, tag="rz")
                    if first:
                        nc.scalar.activation(rz[:, :].rearrange("p (g b) -> p g b", g=2),
                                             xpr, AF.Sigmoid)
                        n_t = GW.tile([128, B], F32, tag="n_t")
                        nc.scalar.activation(n_t[:, :], xpn, AF.Tanh)
                        zn = GW.tile([128, B], F32, tag="zn")
                        nc.vector.tensor_tensor(out=zn[:, :], in0=rz[:, B:2 * B], in1=n_t[:, :], op=OP.mult)
                        nc.vector.tensor_tensor(out=out_h, in0=n_t[:, :], in1=zn[:, :], op=OP.subtract)
                        return
                    ps_rz = PRZ.tile([128, 2 * B], F32)
                    ps_n = PN.tile([128, B], F32)
                    nc.tensor.matmul(ps_rz[:, 0:B], lhsT=w[:, 0:H], rhs=prev, start=True, stop=True)
                    nc.tensor.matmul(ps_rz[:, B:2 * B], lhsT=w[:, H:2 * H], rhs=prev, start=True, stop=True)
                    nc.tensor.matmul(ps_n[:, :], lhsT=w[:, 2 * H:3 * H], rhs=prev, start=True, stop=True)
                    tmp = GW.tile([128, 2 * B], F32, tag="tmp")
                    nc.vector.tensor_tensor(out=tmp[:, :].rearrange("p (g b) -> p g b", g=2),
                                            in0=ps_rz[:, :].rearrange("p (g b) -> p g b", g=2),
                                            in1=xpr, op=OP.add)
                    nc.scalar.activation(rz[:, :], tmp[:, :], AF.Sigmoid)
                    pn = GW.tile([128, B], F32, tag="pn")
                    nc.vector.tensor_tensor(out=pn[:, :], in0=rz[:, 0:B], in1=ps_n[:, :], op=OP.mult)
                    pn2 = GW.tile([128, B], F32, tag="pn2")
                    nc.vector.tensor_tensor(out=pn2[:, :], in0=pn[:, :], in1=xpn, op=OP.add)
                    n_t = GW.tile([128, B], F32, tag="n_t")
                    nc.scalar.activation(n_t[:, :], pn2[:, :], AF.Tanh)
                    dd = GW.tile([128, B], F32, tag="dd")
                    nc.vector.tensor_tensor(out=dd[:, :], in0=prev, in1=n_t[:, :], op=OP.subtract)
                    zd = GW.tile([128, B], F32, tag="zd")
                    nc.vector.tensor_tensor(out=zd[:, :], in0=rz[:, B:2 * B], in1=dd[:, :], op=OP.mult)
                    nc.vector.tensor_tensor(out=out_h, in0=n_t[:, :], in1=zd[:, :], op=OP.add)

                for t in range(T):
                    # fwd: step t writes col t; bwd: scan-step t writes col T-1-t
                    gru_step('f', t - 1, t, first=(t == 0))
                    gru_step('b', T - t, T - 1 - t, first=(t == 0))

            # ============ phase F: emissions ============
            with tc.tile_pool(name="ps_e", bufs=2, space="PSUM") as PE9, \
                 tc.tile_pool(name="ps_t9", bufs=2, space="PSUM") as PT9, \
                 tc.tile_pool(name="sbF", bufs=2) as SF:
                for n in range(2):
                    pse = PE9.tile([L, 512], F32)
                    nc.tensor.matmul(pse[:, :], lhsT=wlin_sb[:, 0, :],
                                     rhs=hT['f'][:, n * 512:(n + 1) * 512], start=True, stop=False)
                    nc.tensor.matmul(pse[:, :], lhsT=wlin_sb[:, 1, :],
                                     rhs=hT['b'][:, n * 512:(n + 1) * 512], start=False, stop=True)
                    nc.vector.tensor_scalar(out=emisT[:, n * 512:(n + 1) * 512], in0=pse[:, :],
                                            scalar1=blin_sb[:, 0:1], scalar2=None, op0=OP.add)
                # token-major copy to DRAM, then strided read back as [B, T, L]
                e_tok = SF.tile([128, JT, L], F32, tag="e_tok")
                for j in range(JT):
                    pst = PT9.tile([128, L], F32)
                    nc.tensor.transpose(out=pst[:, :], in_=emisT[:, j * 128:(j + 1) * 128],
                                        identity=ident[0:L, 0:L])
                    nc.vector.tensor_copy(e_tok[:, j, :], pst[:, :])
                nc.sync.dma_start(out=e_dram[:, :].rearrange("(j p) l -> p j l", p=128),
                                  in_=e_tok[:, :, :])
                nc.sync.dma_start(out=E_dec[:, :, :],
                                  in_=e_dram[:, :].rearrange("(t b) l -> b t l", b=B))

            # ============ phase G: decode scans ============
            with tc.tile_pool(name="ps_d", bufs=2, space="PSUM") as PSD, \
                 tc.tile_pool(name="ps_v", bufs=2, space="PSUM") as PSV, \
                 tc.tile_pool(name="sbG", bufs=3) as SG, \
                 tc.tile_pool(name="crfv", bufs=2) as CV:
                # exp tables
                nc.scalar.activation(expe[:, :], emisT[:, :], AF.Exp)
                nc.scalar.activation(exptr[:, :], trans_sb[:, :], AF.Exp)
                nc.scalar.activation(exps[:, :], startv[:, :], AF.Exp)
                nc.scalar.activation(expend[:, :], endv[:, :], AF.Exp)
                # replicated transition matrices [B, L, L]
                ps_rep = PSD.tile([B, L * L], F32, tag="psd")
                nc.tensor.matmul(ps_rep[:, :], lhsT=ones_row[0:1, 0:B], rhs=trt_row[0:1, :],
                                 start=True, stop=True)
                nc.vector.tensor_copy(TRT_rep[:, :, :].rearrange("b x y -> b (x y)"), ps_rep[:, :])
                ps_rep2 = PSD.tile([B, L * L], F32, tag="psd")
                nc.tensor.matmul(ps_rep2[:, :], lhsT=ones_row[0:1, 0:B], rhs=tr_row[0:1, :],
                                 start=True, stop=True)
                nc.vector.tensor_copy(TR_rep[:, :, :].rearrange("b x y -> b (x y)"), ps_rep2[:, :])
                # A_buf[0] = start + e0 ; V_buf[255] = end + e255
                ps_sr = PSD.tile([B, L], F32, tag="psd")
                nc.tensor.matmul(ps_sr[:, :], lhsT=ones_row[0:1, 0:B], rhs=start_row[0:1, :],
                                 start=True, stop=True)
                nc.vector.tensor_tensor(out=A_buf[:, 0, :], in0=ps_sr[:, :], in1=E_dec[:, 0, :], op=OP.add)
                ps_er = PSD.tile([B, L], F32, tag="psd")
                nc.tensor.matmul(ps_er[:, :], lhsT=ones_row[0:1, 0:B], rhs=end_row[0:1, :],
                                 start=True, stop=True)
                nc.vector.tensor_tensor(out=V_buf[:, T - 1, :], in0=ps_er[:, :], in1=E_dec[:, T - 1, :], op=OP.add)
                # CRF v0 = exp(start) * expe[:, t=0]
                v_cur = CV.tile([L, B], F32, tag="vc")
                nc.vector.tensor_tensor(out=v_cur[:, :], in0=expe[:, 0:B],
                                        in1=exps[:, 0:1].to_broadcast([L, B]), op=OP.mult)

                for k in range(1, T):
                    # viterbi forward (alpha, includes e)
                    sca = SG.tile([B, L, L], F32, tag="sca")
                    nc.vector.tensor_tensor(out=sca[:, :, :],
                                            in0=A_buf[:, k - 1, :].unsqueeze(1).to_broadcast([B, L, L]),
                                            in1=TRT_rep[:, :, :], op=OP.add)
                    ma = SG.tile([B, L], F32, tag="ma")
                    nc.vector.tensor_reduce(out=ma[:, :], in_=sca[:, :, :], axis=AX.X, op=OP.max)
                    nc.vector.tensor_tensor(out=A_buf[:, k, :], in0=ma[:, :], in1=E_dec[:, k, :], op=OP.add)
                    # viterbi backward (v = w + e), t descending
                    tt = T - 1 - k
                    scw = SG.tile([B, L, L], F32, tag="scw")
                    nc.vector.tensor_tensor(out=scw[:, :, :],
                                            in0=V_buf[:, tt + 1, :].unsqueeze(1).to_broadcast([B, L, L]),
                                            in1=TR_rep[:, :, :], op=OP.add)
                    mw = SG.tile([B, L], F32, tag="mw")
                    nc.vector.tensor_reduce(out=mw[:, :], in_=scw[:, :, :], axis=AX.X, op=OP.max)
                    nc.vector.tensor_tensor(out=V_buf[:, tt, :], in0=mw[:, :], in1=E_dec[:, tt, :], op=OP.add)
                    # CRF chain: v <- (exptr.T @ v) * expe[:, t=k]
                    psv = PSV.tile([L, B], F32, tag="psv")
                    nc.tensor.matmul(psv[:, :], lhsT=exptr[:, :], rhs=v_cur[:, :], start=True, stop=True)
                    v_new = CV.tile([L, B], F32, tag="vc")
                    nc.vector.tensor_tensor(out=v_new[:, :], in0=psv[:, :],
                                            in1=expe[:, k * B:(k + 1) * B], op=OP.mult)
                    v_cur = v_new
                    if k % RESC == 0 and k < T - 1:
                        # rescale: v /= max_b, logsc += log(max)
                        ps_vt = PSD.tile([B, L], F32, tag="psd")
                        nc.tensor.transpose(out=ps_vt[:, :], in_=v_cur[:, :], identity=ident[0:L, 0:L])
                        vmax = SG.tile([B, 1], F32, tag="vmax")
                        nc.vector.tensor_reduce(out=vmax[:, :], in_=ps_vt[:, :], axis=AX.X, op=OP.max)
                        rv = SG.tile([B, 1], F32, tag="rv")
                        nc.vector.reciprocal(rv[:, :], vmax[:, :])
                        ps_rvr = PSD.tile([1, B], F32, tag="psd")
                        nc.tensor.transpose(out=ps_rvr[:, :], in_=rv[:, :], identity=ident[0:B, 0:B])
                        rvr_sb = SG.tile([1, B], F32, tag="rvr_sb")
                        nc.scalar.activation(rvr_sb[:, :], ps_rvr[:, :], AF.Copy)
                        logrv = SG.tile([1, B], F32, tag="logrv")
                        nc.scalar.activation(logrv[:, :], ps_rvr[:, :], AF.Ln)
                        nc.vector.tensor_tensor(out=logsc[:, :], in0=logsc[:, :], in1=logrv[:, :],
                                                op=OP.subtract)
                        ps_rep9 = PSV.tile([L, B], F32, tag="psv")
                        nc.tensor.matmul(ps_rep9[:, :], lhsT=ones_row[0:1, 0:L], rhs=rvr_sb[:, :],
                                         start=True, stop=True)
                        v_s = CV.tile([L, B], F32, tag="vc")
                        nc.vector.tensor_tensor(out=v_s[:, :], in0=v_cur[:, :], in1=ps_rep9[:, :], op=OP.mult)
                        v_cur = v_s

                # ---- denominator ----
                vend = SG.tile([L, B], F32, tag="vend")
                nc.vector.tensor_tensor(out=vend[:, :], in0=v_cur[:, :],
                                        in1=expend[:, 0:1].to_broadcast([L, B]), op=OP.mult)
                ps_den = PSD.tile([1, B], F32, tag="psd")
                nc.tensor.matmul(ps_den[:, :], lhsT=ones9[:, :], rhs=vend[:, :], start=True, stop=True)
                logz0 = SG.tile([1, B], F32, tag="logz0")
                nc.scalar.activation(logz0[:, :], ps_den[:, :], AF.Ln)
                den_row = SG.tile([1, B], F32, tag="den_row")
                nc.vector.tensor_tensor(out=den_row[:, :], in0=logz0[:, :], in1=logsc[:, :], op=OP.add)

                # ---- numerator via one-hot labels ----
                for n in range(2):
                    ps_lab = PSV.tile([L, 512], F32, tag="psvBig")
                    nc.tensor.matmul(ps_lab[:, :], lhsT=ones_row[0:1, 0:L],
                                     rhs=lab_row[0:1, n * 512:(n + 1) * 512], start=True, stop=True)
                    nc.vector.tensor_scalar(out=YT[:, n * 512:(n + 1) * 512], in0=ps_lab[:, :],
                                            scalar1=iota9v[:, 0:1], scalar2=None, op0=OP.is_equal)
                EY = SG.tile([L, NTOK], F32, tag="EY")
                nc.vector.tensor_tensor(out=EY[:, :], in0=emisT[:, :], in1=YT[:, :], op=OP.mult)
                NS = SG.tile([L, B], F32, tag="NS")
                nc.vector.tensor_reduce(out=NS[:, :],
                                        in_=EY[:, :].rearrange("p (t b) -> p b t", b=B),
                                        axis=AX.X, op=OP.add)
                QY = SG.tile([L, NTOK - B], F32, tag="QY")
                for n in range(2):
                    ps_q = PSV.tile([L, 512], F32, tag="psvBig")
                    nc.tensor.matmul(ps_q[:, :], lhsT=trans_sb[:, :],
                                     rhs=YT[:, n * 512:(n + 1) * 512], start=True, stop=True)
                    if n == 0:
                        nc.vector.tensor_tensor(out=QY[:, 0:512], in0=ps_q[:, :],
                                                in1=YT[:, B:512 + B], op=OP.mult)
                    else:
                        nc.vector.tensor_tensor(out=QY[:, 512:NTOK - B], in0=ps_q[:, 0:512 - B],
                                                in1=YT[:, 512 + B:NTOK], op=OP.mult)
                TSc = SG.tile([L, B], F32, tag="TSc")
                nc.vector.tensor_reduce(out=TSc[:, :],
                                        in_=QY[:, :].rearrange("p (t b) -> p b t", b=B),
                                        axis=AX.X, op=OP.add)
                SEs = SG.tile([L, B], F32, tag="SEs")
                nc.vector.tensor_tensor(out=SEs[:, :], in0=YT[:, 0:B],
                                        in1=startv[:, 0:1].to_broadcast([L, B]), op=OP.mult)
                Ee = SG.tile([L, B], F32, tag="Ee")
                nc.vector.tensor_tensor(out=Ee[:, :], in0=YT[:, NTOK - B:NTOK],
                                        in1=endv[:, 0:1].to_broadcast([L, B]), op=OP.mult)
                nc.vector.tensor_tensor(out=NS[:, :], in0=NS[:, :], in1=TSc[:, :], op=OP.add)
                nc.vector.tensor_tensor(out=NS[:, :], in0=NS[:, :], in1=SEs[:, :], op=OP.add)
                nc.vector.tensor_tensor(out=NS[:, :], in0=NS[:, :], in1=Ee[:, :], op=OP.add)
                ps_num = PSD.tile([1, B], F32, tag="psd")
                nc.tensor.matmul(ps_num[:, :], lhsT=ones9[:, :], rhs=NS[:, :], start=True, stop=True)
                llh_sb = SG.tile([1, 4 * B], F32, tag="llh_sb")
                nc.vector.tensor_tensor(out=llh_sb[:, 0:B], in0=ps_num[:, :], in1=den_row[:, :],
                                        op=OP.subtract)
                nc.vector.tensor_copy(llh_sb[:, B:2 * B], ps_num[:, :])
                nc.vector.tensor_copy(llh_sb[:, 2 * B:3 * B], den_row[:, :])
                nc.vector.tensor_copy(llh_sb[:, 3 * B:4 * B], logsc[:, :])
                nc.sync.dma_start(out=ostat_d[:, :], in_=llh_sb[:, :])

                # ---- gamma = A + V - E, argmax over labels ----
                gamma = SG.tile([B, T, L], F32, tag="gamma")
                nc.vector.tensor_tensor(out=gamma[:, :, :], in0=A_buf[:, :, :], in1=V_buf[:, :, :], op=OP.add)
                nc.vector.tensor_tensor(out=gamma[:, :, :], in0=gamma[:, :, :], in1=E_dec[:, :, :],
                                        op=OP.subtract)
                gmax = SG.tile([B, T], F32, tag="gmax")
                nc.vector.tensor_reduce(out=gmax[:, :], in_=gamma[:, :, :], axis=AX.X, op=OP.max)
                msk = SG.tile([B, T, L], F32, tag="msk")
                nc.vector.tensor_tensor(out=msk[:, :, :], in0=gamma[:, :, :],
                                        in1=gmax[:, :].unsqueeze(2).to_broadcast([B, T, L]), op=OP.is_equal)
                nc.vector.tensor_tensor(out=msk[:, :, :], in0=msk[:, :, :],
                                        in1=iota9f[:, :].unsqueeze(1).to_broadcast([B, T, L]), op=OP.mult)
                dec_f = SG.tile([B, T], F32, tag="dec_f")
                nc.vector.tensor_reduce(out=dec_f[:, :], in_=msk[:, :, :], axis=AX.X, op=OP.max)
                dec_i = SG.tile([B, T], I32, tag="dec_i")
                nc.vector.tensor_copy(dec_i[:, :], dec_f[:, :])
                nc.sync.dma_start(out=odec_d[:, :], in_=dec_i[:, :])
    nc.compile()
    return nc


_cache = {}


def kernel(**inputs):
    if 'nc' not in _cache:
        _cache['nc'] = build_nc()
    nc = _cache['nc']

    f32 = lambda k: np.ascontiguousarray(np.asarray(inputs[k], dtype=np.float32))
    input_ids = np.asarray(inputs['input_ids'], dtype=np.int32)
    labels = np.asarray(inputs['labels'], dtype=np.int32)
    shared = {
        'emb': f32('emb'),
        'wihT_f': np.ascontiguousarray(f32('Wih_f').T), 'wihT_b': np.ascontiguousarray(f32('Wih_b').T),
        'whhT_f': np.ascontiguousarray(f32('Whh_f').T), 'whhT_b': np.ascontiguousarray(f32('Whh_b').T),
        'bih_f': f32('bih_f'), 'bih_b': f32('bih_b'),
        'bhh_f': f32('bhh_f'), 'bhh_b': f32('bhh_b'),
        'wlinT': np.ascontiguousarray(f32('Wlin').T),
        'blin': f32('blin'), 'trans': f32('trans'), 'start': f32('start'), 'end': f32('end'),
    }
    in_maps = []
    for c in range(NCORES):
        bs = slice(c * B, (c + 1) * B)
        ids_flat = np.ascontiguousarray(input_ids[bs].T).reshape(-1)      # token = t*B+b
        ids_perm = np.ascontiguousarray(ids_flat.reshape(NTOK // 128, 128).T)  # [128, 8]
        lab_flat = np.ascontiguousarray(labels[bs].T).reshape(1, -1).astype(np.float32)
        in_maps.append({**shared, 'ids': ids_perm, 'lab': lab_flat})

    res = run_bass_kernel_spmd(nc, in_maps, core_ids=list(range(NCORES)),
                               trace=bool(os.environ.get('KERNEL_TRACE')))
    _cache['last_result'] = res
    dec = np.concatenate([res.results[c]['out_dec'] for c in range(NCORES)], axis=0).astype(np.int32)
    llh = np.concatenate([res.results[c]['out_stat'].reshape(-1)[:B] for c in range(NCORES)])
    loss = np.float32(-np.mean(llh))
    return dec, loss


# revision 14
# speedup vs baseline: 1.5852x; 1.5852x over previous
"""BiGRU+CRF NER kernel for 8 TRN2 NeuronCores.

Sharding: data-parallel over batch (B=32 -> 4 per core). Each core runs:
  embedding gather (indirect DMA) -> x transpose (PE) -> xp = x@Wih.T (PE)
  -> fwd+bwd GRU scans (interleaved, transposed state layout [H=128, B])
  -> emissions -> Viterbi decode via max-marginals (fwd+bwd max-plus scans,
  no backtrace) -> CRF log-likelihood via exp-domain matmul chain.
Host only shards/permutes inputs and concatenates outputs.
"""
import os
import sys

sys.path.insert(0, '/opt/trn_rl_repo')
import numpy as np
import types
try:
    import antenv.axon_hooks  # noqa: F401
except Exception:
    import antenv
    _m = types.ModuleType('antenv.axon_hooks')
    _m.get_axon_ntff_profile_hook = lambda: None
    sys.modules['antenv.axon_hooks'] = _m

import concourse.bass as bass
import concourse.bacc as bacc
import concourse.mybir as mybir
from concourse import tile
from concourse.bass_utils import run_bass_kernel_spmd

F32 = mybir.dt.float32
I32 = mybir.dt.int32
AF = mybir.ActivationFunctionType
OP = mybir.AluOpType
AX = mybir.AxisListType

VOCAB, HID, H, T, L = 21128, 768, 128, 256, 9
B = 4                  # local batch per core
NTOK = B * T           # 1024 tokens per core, token index = t*B + b
NCORES = 8
RESC = 16              # CRF rescale interval (bulk Ln of stored maxes at end)


def build_nc():
    nc = bacc.Bacc(None, target_bir_lowering=False)
    ids_d = nc.declare_dram_parameter("ids", [128, NTOK // 128], I32, isOutput=False)
    lab_d = nc.declare_dram_parameter("lab", [1, NTOK], F32, isOutput=False)
    emb_d = nc.declare_dram_parameter("emb", [VOCAB, HID], F32, isOutput=False)
    wih_d = {d: nc.declare_dram_parameter(f"wihT_{d}", [HID, 3 * H], F32, isOutput=False) for d in "fb"}
    whh_d = {d: nc.declare_dram_parameter(f"whhT_{d}", [H, 3 * H], F32, isOutput=False) for d in "fb"}
    bih_d = {d: nc.declare_dram_parameter(f"bih_{d}", [3 * H], F32, isOutput=False) for d in "fb"}
    bhh_d = {d: nc.declare_dram_parameter(f"bhh_{d}", [3 * H], F32, isOutput=False) for d in "fb"}
    wlin_d = nc.declare_dram_parameter("wlinT", [2 * H, L], F32, isOutput=False)
    blin_d = nc.declare_dram_parameter("blin", [L], F32, isOutput=False)
    trans_d = nc.declare_dram_parameter("trans", [L, L], F32, isOutput=False)
    start_d = nc.declare_dram_parameter("start", [L], F32, isOutput=False)
    end_d = nc.declare_dram_parameter("end", [L], F32, isOutput=False)
    odec_d = nc.declare_dram_parameter("out_dec", [B, T], I32, isOutput=True)
    ostat_d = nc.declare_dram_parameter("out_stat", [1, 4 * B], F32, isOutput=True)
    e_dram = nc.dram_tensor("e_scratch", [NTOK, L], F32)

    JT = NTOK // 128  # 8 token tiles

    with tile.TileContext(nc) as tc:
        with tc.tile_pool(name="persist", bufs=1) as P:
            # ---- persistent tiles ----
            ident = P.tile([128, 128], F32, tag="ident")
            ones_row = P.tile([1, 128], F32, tag="ones_row")
            ones9 = P.tile([L, 1], F32, tag="ones9")
            iota9f = P.tile([B, L], F32, tag="iota9f")
            iota9v = P.tile([L, 1], F32, tag="iota9v")
            whh_sb = {d: P.tile([H, 3 * H], F32, tag=f"whh_{d}", name=f"whh_{d}") for d in "fb"}
            wlin_sb = P.tile([H, 2, L], F32, tag="wlin_sb")
            blin_sb = P.tile([L, 1], F32, tag="blin_sb")
            trans_sb = P.tile([L, L], F32, tag="trans_sb")
            trt_row = P.tile([1, L * L], F32, tag="trt_row")
            tr_row = P.tile([1, L * L], F32, tag="tr_row")
            startv = P.tile([L, 1], F32, tag="startv")
            endv = P.tile([L, 1], F32, tag="endv")
            start_row = P.tile([1, L], F32, tag="start_row")
            end_row = P.tile([1, L], F32, tag="end_row")
            xp = {d: P.tile([128, 3, NTOK], F32, tag=f"xp_{d}", name=f"xp_{d}") for d in "fb"}
            hT = {d: P.tile([128, NTOK], F32, tag=f"hT_{d}", name=f"hT_{d}") for d in "fb"}
            emisT = P.tile([L, NTOK], F32, tag="emisT")
            E8 = P.tile([32 + B, T, L], F32, tag="E8")
            AV8 = P.tile([32 + B, T, L], F32, tag="AV8")
            TR8 = P.tile([32 + B, L, L], F32, tag="TR8")
            M_burst = P.tile([B, 16], F32, tag="M_burst")
            TRT_rep = P.tile([B, L, L], F32, tag="TRT_rep")
            TR_rep = P.tile([B, L, L], F32, tag="TR_rep")
            expe = P.tile([L, NTOK], F32, tag="expe")
            exptr = P.tile([L, L], F32, tag="exptr")
            exps = P.tile([L, 1], F32, tag="exps")
            expend = P.tile([L, 1], F32, tag="expend")
            logsc = P.tile([1, B], F32, tag="logsc")
            lab_row = P.tile([1, NTOK], F32, tag="lab_row")
            YT = P.tile([L, NTOK], F32, tag="YT")

            # ================= phase A: consts + weights =================
            with tc.tile_pool(name="sbA", bufs=1) as SA, \
                 tc.tile_pool(name="psA", bufs=2, space="PSUM") as PSA, \
                 tc.tile_pool(name="psX", bufs=2, space="PSUM") as PSX, \
                 tc.tile_pool(name="cpA", bufs=4) as CPA:
                ids_sb = SA.tile([128, JT], I32, tag="ids_sb")
                nc.sync.dma_start(out=ids_sb[:, :], in_=ids_d[:, :])
                nc.sync.dma_start(out=lab_row[:, :], in_=lab_d[:, :])
                wih_sb = {d: SA.tile([128, HID // 128, 3 * H], F32, tag=f"wih_{d}", name=f"wih_{d}") for d in "fb"}
                for d in "fb":
                    nc.sync.dma_start(out=wih_sb[d][:, :, :],
                                      in_=wih_d[d][:, :].rearrange("(c p) n -> p c n", p=128))
                    nc.sync.dma_start(out=whh_sb[d][:, :], in_=whh_d[d][:, :])
                badd = {}
                for d in "fb":
                    bi = SA.tile([128, 3], F32, tag=f"bi_{d}")
                    bh = SA.tile([128, 3], F32, tag=f"bh_{d}")
                    nc.sync.dma_start(out=bi[:, :], in_=bih_d[d][:].rearrange("(g p) -> p g", p=128))
                    nc.sync.dma_start(out=bh[:, :], in_=bhh_d[d][:].rearrange("(g p) -> p g", p=128))
                    ba = SA.tile([128, 3], F32, tag=f"ba_{d}")
                    nc.vector.tensor_tensor(out=ba[:, :], in0=bi[:, :], in1=bh[:, :], op=OP.add)
                    badd[d] = ba
                nc.sync.dma_start(out=wlin_sb[:, :, :],
                                  in_=wlin_d[:, :].rearrange("(h p) n -> p h n", p=128))
                nc.sync.dma_start(out=blin_sb[:, :], in_=blin_d[:].unsqueeze(1))
                nc.sync.dma_start(out=trans_sb[:, :], in_=trans_d[:, :])
                nc.sync.dma_start(out=tr_row[0:1, :].rearrange("o (i j) -> o i j", i=L),
                                  in_=trans_d[:, :].unsqueeze(0))
                nc.sync.dma_start(out=trt_row[0:1, :].rearrange("o (j i) -> o j i", j=L),
                                  in_=trans_d[:, :].transpose([1, 0]).unsqueeze(0))
                nc.sync.dma_start(out=startv[:, :], in_=start_d[:].unsqueeze(1))
                nc.sync.dma_start(out=endv[:, :], in_=end_d[:].unsqueeze(1))
                nc.sync.dma_start(out=start_row[:, :], in_=start_d[:].unsqueeze(0))
                nc.sync.dma_start(out=end_row[:, :], in_=end_d[:].unsqueeze(0))
                nc.vector.memset(ones_row[:, :], 1.0)
                nc.vector.memset(ones9[:, :], 1.0)
                nc.vector.memset(logsc[:, :], 0.0)
                # identity matrix + iotas
                ia_i = SA.tile([128, 128], I32, tag="ia_i")
                ib_i = SA.tile([128, 1], I32, tag="ib_i")
                nc.gpsimd.iota(ia_i[:, :], pattern=[[1, 128]], channel_multiplier=0)
                nc.gpsimd.iota(ib_i[:, :], pattern=[[0, 1]], channel_multiplier=1)
                ia_f = SA.tile([128, 128], F32, tag="ia_f")
                ib_f = SA.tile([128, 1], F32, tag="ib_f")
                nc.vector.tensor_copy(ia_f[:, :], ia_i[:, :])
                nc.vector.tensor_copy(ib_f[:, :], ib_i[:, :])
                nc.vector.tensor_tensor(out=ident[:, :], in0=ia_f[:, :],
                                        in1=ib_f[:, 0:1].to_broadcast([128, 128]), op=OP.is_equal)
                i9 = SA.tile([B, L], I32, tag="i9")
                nc.gpsimd.iota(i9[:, :], pattern=[[1, L]], channel_multiplier=0)
                nc.vector.tensor_copy(iota9f[:, :], i9[:, :])
                i9v = SA.tile([L, 1], I32, tag="i9v")
                nc.gpsimd.iota(i9v[:, :], pattern=[[0, 1]], channel_multiplier=1)
                nc.vector.tensor_copy(iota9v[:, :], i9v[:, :])

                # ============ phase B: embedding gather ============
                x_sb = SA.tile([128, JT, HID], F32, tag="x_sb")
                for j in range(JT):
                    nc.gpsimd.indirect_dma_start(
                        out=x_sb[:, j, :], out_offset=None,
                        in_=emb_d[:, :],
                        in_offset=bass.IndirectOffsetOnAxis(ap=ids_sb[:, j:j + 1], axis=0))

                # ============ phase C: transpose x ============
                xT = SA.tile([128, HID // 128, NTOK], F32, tag="xT")
                k = 0
                for j in range(JT):
                    for c in range(HID // 128):
                        pst = PSA.tile([128, 128], F32)
                        nc.tensor.transpose(out=pst[:, :], in_=x_sb[:, j, c * 128:(c + 1) * 128],
                                            identity=ident[:, :])
                        dst = xT[:, c, j * 128:(j + 1) * 128]
                        if k % 2 == 0:
                            nc.vector.tensor_copy(dst, pst[:, :])
                        else:
                            nc.scalar.activation(dst, pst[:, :], AF.Copy)
                        k += 1

                # ============ phase D: xp = x @ WihT + (bih+bhh) ============
                for d in "fb":
                    for g in range(3):
                        for n in range(2):
                            psx = PSX.tile([128, 512], F32)
                            for c in range(HID // 128):
                                nc.tensor.matmul(psx[:, :],
                                                 lhsT=wih_sb[d][:, c, g * H:(g + 1) * H],
                                                 rhs=xT[:, c, n * 512:(n + 1) * 512],
                                                 start=(c == 0), stop=(c == HID // 128 - 1))
                            nc.vector.tensor_scalar(
                                out=xp[d][:, g, n * 512:(n + 1) * 512], in0=psx[:, :],
                                scalar1=badd[d][:, g:g + 1], scalar2=None, op0=OP.add)

            # ============ phase E: GRU scans (fwd & bwd interleaved) ============
            with tc.tile_pool(name="ps_rz", bufs=3, space="PSUM") as PRZ, \
                 tc.tile_pool(name="ps_n", bufs=3, space="PSUM") as PN, \
                 tc.tile_pool(name="gruw", bufs=4) as GW:
                def gru_step(d, tprev_col, t_col, first):
                    w = whh_sb[d]
                    prev = hT[d][:, tprev_col * B:(tprev_col + 1) * B] if not first else None
                    xpr = xp[d][:, 0:2, t_col * B:(t_col + 1) * B]  # [128,2,B]
                    xpn = xp[d][:, 2, t_col * B:(t_col + 1) * B]    # [128,B]
                    out_h = hT[d][:, t_col * B:(t_col + 1) * B]
                    rz = GW.tile([128, 2 * B], F32, tag="rz")
                    if first:
                        nc.scalar.activation(rz[:, :].rearrange("p (g b) -> p g b", g=2),
                                             xpr, AF.Sigmoid)
                        n_t = GW.tile([128, B], F32, tag="n_t")
                        nc.scalar.activation(n_t[:, :], xpn, AF.Tanh)
                        zn = GW.tile([128, B], F32, tag="zn")
                        nc.vector.tensor_tensor(out=zn[:, :], in0=rz[:, B:2 * B], in1=n_t[:, :], op=OP.mult)
                        nc.vector.tensor_tensor(out=out_h, in0=n_t[:, :], in1=zn[:, :], op=OP.subtract)
                        return
                    ps_rz = PRZ.tile([128, 2 * B], F32)
                    ps_n = PN.tile([128, B], F32)
                    nc.tensor.matmul(ps_rz[:, 0:B], lhsT=w[:, 0:H], rhs=prev, start=True, stop=True)
                    nc.tensor.matmul(ps_rz[:, B:2 * B], lhsT=w[:, H:2 * H], rhs=prev, start=True, stop=True)
                    nc.tensor.matmul(ps_n[:, :], lhsT=w[:, 2 * H:3 * H], rhs=prev, start=True, stop=True)
                    tmp = GW.tile([128, 2 * B], F32, tag="tmp")
                    nc.vector.tensor_tensor(out=tmp[:, :].rearrange("p (g b) -> p g b", g=2),
                                            in0=ps_rz[:, :].rearrange("p (g b) -> p g b", g=2),
                                            in1=xpr, op=OP.add)
                    nc.scalar.activation(rz[:, :], tmp[:, :], AF.Sigmoid)
                    pn = GW.tile([128, B], F32, tag="pn")
                    nc.vector.tensor_tensor(out=pn[:, :], in0=rz[:, 0:B], in1=ps_n[:, :], op=OP.mult)
                    pn2 = GW.tile([128, B], F32, tag="pn2")
                    nc.vector.tensor_tensor(out=pn2[:, :], in0=pn[:, :], in1=xpn, op=OP.add)
                    n_t = GW.tile([128, B], F32, tag="n_t")
                    nc.scalar.activation(n_t[:, :], pn2[:, :], AF.Tanh)
                    dd = GW.tile([128, B], F32, tag="dd")
                    nc.vector.tensor_tensor(out=dd[:, :], in0=prev, in1=n_t[:, :], op=OP.subtract)
                    zd = GW.tile([128, B], F32, tag="zd")
                    nc.vector.tensor_tensor(out=zd[:, :], in0=rz[:, B:2 * B], in1=dd[:, :], op=OP.mult)
                    nc.vector.tensor_tensor(out=out_h, in0=n_t[:, :], in1=zd[:, :], op=OP.add)

                for t in range(T):
                    # fwd: step t writes col t; bwd: scan-step t writes col T-1-t
                    gru_step('f', t - 1, t, first=(t == 0))
                    gru_step('b', T - t, T - 1 - t, first=(t == 0))

            # ============ phase F: emissions ============
            with tc.tile_pool(name="ps_e", bufs=2, space="PSUM") as PE9, \
                 tc.tile_pool(name="ps_t9", bufs=2, space="PSUM") as PT9, \
                 tc.tile_pool(name="sbF", bufs=2) as SF:
                for n in range(2):
                    pse = PE9.tile([L, 512], F32)
                    nc.tensor.matmul(pse[:, :], lhsT=wlin_sb[:, 0, :],
                                     rhs=hT['f'][:, n * 512:(n + 1) * 512], start=True, stop=False)
                    nc.tensor.matmul(pse[:, :], lhsT=wlin_sb[:, 1, :],
                                     rhs=hT['b'][:, n * 512:(n + 1) * 512], start=False, stop=True)
                    nc.vector.tensor_scalar(out=emisT[:, n * 512:(n + 1) * 512], in0=pse[:, :],
                                            scalar1=blin_sb[:, 0:1], scalar2=None, op0=OP.add)
                # token-major copy to DRAM, then strided read back as [B, T, L]
                e_tok = SF.tile([128, JT, L], F32, tag="e_tok")
                for j in range(JT):
                    pst = PT9.tile([128, L], F32)
                    nc.tensor.transpose(out=pst[:, :], in_=emisT[:, j * 128:(j + 1) * 128],
                                        identity=ident[0:L, 0:L])
                    nc.vector.tensor_copy(e_tok[:, j, :], pst[:, :])
                nc.sync.dma_start(out=e_dram[:, :].rearrange("(j p) l -> p j l", p=128),
                                  in_=e_tok[:, :, :])
                nc.sync.dma_start(out=E8[0:B, :, :],
                                  in_=e_dram[:, :].rearrange("(t b) l -> b t l", b=B))
                nc.sync.dma_start(out=E8[32:32 + B, :, :],
                                  in_=e_dram[:, :].rearrange("(t b) l -> b t l", b=B))

            # ============ phase G: decode scans ============
            with tc.tile_pool(name="ps_d", bufs=2, space="PSUM") as PSD, \
                 tc.tile_pool(name="ps_v", bufs=2, space="PSUM") as PSV, \
                 tc.tile_pool(name="sbG", bufs=3) as SG, \
                 tc.tile_pool(name="crfv", bufs=2) as CV:
                # exp tables
                nc.scalar.activation(expe[:, :], emisT[:, :], AF.Exp)
                nc.scalar.activation(exptr[:, :], trans_sb[:, :], AF.Exp)
                nc.scalar.activation(exps[:, :], startv[:, :], AF.Exp)
                nc.scalar.activation(expend[:, :], endv[:, :], AF.Exp)
                # replicated transition matrices [B, L, L]
                ps_rep = PSD.tile([B, L * L], F32, tag="psd")
                nc.tensor.matmul(ps_rep[:, :], lhsT=ones_row[0:1, 0:B], rhs=trt_row[0:1, :],
                                 start=True, stop=True)
                nc.vector.tensor_copy(TRT_rep[:, :, :].rearrange("b x y -> b (x y)"), ps_rep[:, :])
                ps_rep2 = PSD.tile([B, L * L], F32, tag="psd")
                nc.tensor.matmul(ps_rep2[:, :], lhsT=ones_row[0:1, 0:B], rhs=tr_row[0:1, :],
                                 start=True, stop=True)
                nc.vector.tensor_copy(TR_rep[:, :, :].rearrange("b x y -> b (x y)"), ps_rep2[:, :])
                nc.vector.tensor_copy(TR8[0:B, :, :].rearrange("b x y -> b (x y)"), ps_rep[:, :])
                nc.sync.dma_start(out=TR8[32:32 + B, :, :],
                                  in_=TR_rep[:, :, :])
                nc.vector.memset(M_burst[:, :], 1.0)
                # A_buf[0] = start + e0 ; V_buf[255] = end + e255
                ps_sr = PSD.tile([B, L], F32, tag="psd")
                nc.tensor.matmul(ps_sr[:, :], lhsT=ones_row[0:1, 0:B], rhs=start_row[0:1, :],
                                 start=True, stop=True)
                nc.vector.tensor_tensor(out=AV8[0:B, 0, :], in0=ps_sr[:, :], in1=E8[0:B, 0, :], op=OP.add)
                ps_er = PSD.tile([B, L], F32, tag="psd")
                nc.tensor.matmul(ps_er[:, :], lhsT=ones_row[0:1, 0:B], rhs=end_row[0:1, :],
                                 start=True, stop=True)
                tmpv = SG.tile([B, L], F32, tag="tmpv")
                nc.vector.tensor_tensor(out=tmpv[:, :], in0=ps_er[:, :], in1=E8[0:B, T - 1, :], op=OP.add)
                nc.sync.dma_start(out=AV8[32:32 + B, 0, :], in_=tmpv[:, :])
                # CRF v0 = exp(start) * expe[:, t=0]
                v_cur = CV.tile([L, B], F32, tag="vc")
                nc.vector.tensor_tensor(out=v_cur[:, :], in0=expe[:, 0:B],
                                        in1=exps[:, 0:1].to_broadcast([L, B]), op=OP.mult)

                for k in range(1, T):
                    # merged viterbi fwd (rows 0:B) + bwd (rows B:2B, scan-indexed)
                    sca = SG.tile([32 + B, L, L], F32, tag="sca")
                    nc.vector.tensor_tensor(out=sca[:, :, :],
                                            in0=AV8[:, k - 1, :].unsqueeze(1).to_broadcast([32 + B, L, L]),
                                            in1=TR8[:, :, :], op=OP.add)
                    ma = SG.tile([32 + B, L], F32, tag="ma")
                    nc.vector.tensor_reduce(out=ma[:, :], in_=sca[:, :, :], axis=AX.X, op=OP.max)
                    nc.vector.tensor_tensor(out=AV8[0:B, k, :], in0=ma[0:B, :], in1=E8[0:B, k, :], op=OP.add)
                    nc.vector.tensor_tensor(out=AV8[32:32 + B, k, :], in0=ma[32:32 + B, :],
                                            in1=E8[32:32 + B, T - 1 - k, :], op=OP.add)
                    # CRF chain: v <- (exptr.T @ v) * expe[:, t=k]
                    psv = PSV.tile([L, B], F32, tag="psv")
                    nc.tensor.matmul(psv[:, :], lhsT=exptr[:, :], rhs=v_cur[:, :], start=True, stop=True)
                    v_new = CV.tile([L, B], F32, tag="vc")
                    nc.vector.tensor_tensor(out=v_new[:, :], in0=psv[:, :],
                                            in1=expe[:, k * B:(k + 1) * B], op=OP.mult)
                    v_cur = v_new
                    if k % RESC == 0 and k < T - 1:
                        # rescale: v /= max_b, logsc += log(max)
                        ps_vt = PSD.tile([B, L], F32, tag="psd")
                        nc.tensor.transpose(out=ps_vt[:, :], in_=v_cur[:, :], identity=ident[0:L, 0:L])
                        bi_idx = k // RESC - 1
                        nc.vector.tensor_reduce(out=M_burst[:, bi_idx:bi_idx + 1], in_=ps_vt[:, :],
                                                axis=AX.X, op=OP.max)
                        rv = SG.tile([B, 1], F32, tag="rv")
                        nc.vector.reciprocal(rv[:, :], M_burst[:, bi_idx:bi_idx + 1])
                        ps_rvr = PSD.tile([1, B], F32, tag="psd")
                        nc.tensor.transpose(out=ps_rvr[:, :], in_=rv[:, :], identity=ident[0:B, 0:B])
                        rvr_sb = SG.tile([1, B], F32, tag="rvr_sb")
                        nc.scalar.activation(rvr_sb[:, :], ps_rvr[:, :], AF.Copy)
                        ps_rep9 = PSV.tile([L, B], F32, tag="psv")
                        nc.tensor.matmul(ps_rep9[:, :], lhsT=ones_row[0:1, 0:L], rhs=rvr_sb[:, :],
                                         start=True, stop=True)
                        v_s = CV.tile([L, B], F32, tag="vc")
                        nc.vector.tensor_tensor(out=v_s[:, :], in0=v_cur[:, :], in1=ps_rep9[:, :], op=OP.mult)
                        v_cur = v_s

                # ---- denominator ----
                vend = SG.tile([L, B], F32, tag="vend")
                nc.vector.tensor_tensor(out=vend[:, :], in0=v_cur[:, :],
                                        in1=expend[:, 0:1].to_broadcast([L, B]), op=OP.mult)
                ps_den = PSD.tile([1, B], F32, tag="psd")
                nc.tensor.matmul(ps_den[:, :], lhsT=ones9[:, :], rhs=vend[:, :], start=True, stop=True)
                logz0 = SG.tile([1, B], F32, tag="logz0")
                nc.scalar.activation(logz0[:, :], ps_den[:, :], AF.Ln)
                lnm = SG.tile([B, 16], F32, tag="lnm")
                nc.scalar.activation(lnm[:, :], M_burst[:, :], AF.Ln)
                lsc_col = SG.tile([B, 1], F32, tag="lsc_col")
                nc.vector.tensor_reduce(out=lsc_col[:, :], in_=lnm[:, :], axis=AX.X, op=OP.add)
                ps_lsc = PSD.tile([1, B], F32, tag="psd")
                nc.tensor.transpose(out=ps_lsc[:, :], in_=lsc_col[:, :], identity=ident[0:B, 0:B])
                nc.vector.tensor_copy(logsc[:, :], ps_lsc[:, :])
                den_row = SG.tile([1, B], F32, tag="den_row")
                nc.vector.tensor_tensor(out=den_row[:, :], in0=logz0[:, :], in1=logsc[:, :], op=OP.add)

                # ---- numerator via one-hot labels ----
                for n in range(2):
                    ps_lab = PSV.tile([L, 512], F32, tag="psvBig")
                    nc.tensor.matmul(ps_lab[:, :], lhsT=ones_row[0:1, 0:L],
                                     rhs=lab_row[0:1, n * 512:(n + 1) * 512], start=True, stop=True)
                    nc.vector.tensor_scalar(out=YT[:, n * 512:(n + 1) * 512], in0=ps_lab[:, :],
                                            scalar1=iota9v[:, 0:1], scalar2=None, op0=OP.is_equal)
                EY = SG.tile([L, NTOK], F32, tag="EY")
                nc.vector.tensor_tensor(out=EY[:, :], in0=emisT[:, :], in1=YT[:, :], op=OP.mult)
                NS = SG.tile([L, B], F32, tag="NS")
                nc.vector.tensor_reduce(out=NS[:, :],
                                        in_=EY[:, :].rearrange("p (t b) -> p b t", b=B),
                                        axis=AX.X, op=OP.add)
                QY = SG.tile([L, NTOK - B], F32, tag="QY")
                for n in range(2):
                    ps_q = PSV.tile([L, 512], F32, tag="psvBig")
                    nc.tensor.matmul(ps_q[:, :], lhsT=trans_sb[:, :],
                                     rhs=YT[:, n * 512:(n + 1) * 512], start=True, stop=True)
                    if n == 0:
                        nc.vector.tensor_tensor(out=QY[:, 0:512], in0=ps_q[:, :],
                                                in1=YT[:, B:512 + B], op=OP.mult)
                    else:
                        nc.vector.tensor_tensor(out=QY[:, 512:NTOK - B], in0=ps_q[:, 0:512 - B],
                                                in1=YT[:, 512 + B:NTOK], op=OP.mult)
                TSc = SG.tile([L, B], F32, tag="TSc")
                nc.vector.tensor_reduce(out=TSc[:, :],
                                        in_=QY[:, :].rearrange("p (t b) -> p b t", b=B),
                                        axis=AX.X, op=OP.add)
                SEs = SG.tile([L, B], F32, tag="SEs")
                nc.vector.tensor_tensor(out=SEs[:, :], in0=YT[:, 0:B],
                                        in1=startv[:, 0:1].to_broadcast([L, B]), op=OP.mult)
                Ee = SG.tile([L, B], F32, tag="Ee")
                nc.vector.tensor_tensor(out=Ee[:, :], in0=YT[:, NTOK - B:NTOK],
                                        in1=endv[:, 0:1].to_broadcast([L, B]), op=OP.mult)
                nc.vector.tensor_tensor(out=NS[:, :], in0=NS[:, :], in1=TSc[:, :], op=OP.add)
                nc.vector.tensor_tensor(out=NS[:, :], in0=NS[:, :], in1=SEs[:, :], op=OP.add)
                nc.vector.tensor_tensor(out=NS[:, :], in0=NS[:, :], in1=Ee[:, :], op=OP.add)
                ps_num = PSD.tile([1, B], F32, tag="psd")
                nc.tensor.matmul(ps_num[:, :], lhsT=ones9[:, :], rhs=NS[:, :], start=True, stop=True)
                llh_sb = SG.tile([1, 4 * B], F32, tag="llh_sb")
                nc.vector.tensor_tensor(out=llh_sb[:, 0:B], in0=ps_num[:, :], in1=den_row[:, :],
                                        op=OP.subtract)
                nc.vector.tensor_copy(llh_sb[:, B:2 * B], ps_num[:, :])
                nc.vector.tensor_copy(llh_sb[:, 2 * B:3 * B], den_row[:, :])
                nc.vector.tensor_copy(llh_sb[:, 3 * B:4 * B], logsc[:, :])
                nc.sync.dma_start(out=ostat_d[:, :], in_=llh_sb[:, :])

                # ---- gamma = A + V - E, argmax over labels ----
                Vlow = SG.tile([B, T, L], F32, tag="Vlow")
                nc.sync.dma_start(out=Vlow[:, :, :], in_=AV8[32:32 + B, :, :])
                gamma = SG.tile([B, T, L], F32, tag="gamma")
                for t in range(T):
                    nc.vector.tensor_tensor(out=gamma[:, t, :], in0=AV8[0:B, t, :],
                                            in1=Vlow[:, T - 1 - t, :], op=OP.add)
                nc.vector.tensor_tensor(out=gamma[:, :, :], in0=gamma[:, :, :], in1=E8[0:B, :, :],
                                        op=OP.subtract)
                gmax = SG.tile([B, T], F32, tag="gmax")
                nc.vector.tensor_reduce(out=gmax[:, :], in_=gamma[:, :, :], axis=AX.X, op=OP.max)
                msk = SG.tile([B, T, L], F32, tag="msk")
                nc.vector.tensor_tensor(out=msk[:, :, :], in0=gamma[:, :, :],
                                        in1=gmax[:, :].unsqueeze(2).to_broadcast([B, T, L]), op=OP.is_equal)
                nc.vector.tensor_tensor(out=msk[:, :, :], in0=msk[:, :, :],
                                        in1=iota9f[:, :].unsqueeze(1).to_broadcast([B, T, L]), op=OP.mult)
                dec_f = SG.tile([B, T], F32, tag="dec_f")
                nc.vector.tensor_reduce(out=dec_f[:, :], in_=msk[:, :, :], axis=AX.X, op=OP.max)
                dec_i = SG.tile([B, T], I32, tag="dec_i")
                nc.vector.tensor_copy(dec_i[:, :], dec_f[:, :])
                nc.sync.dma_start(out=odec_d[:, :], in_=dec_i[:, :])
    nc.compile()
    return nc


_cache = {}


def kernel(**inputs):
    if 'nc' not in _cache:
        _cache['nc'] = build_nc()
    nc = _cache['nc']

    f32 = lambda k: np.ascontiguousarray(np.asarray(inputs[k], dtype=np.float32))
    input_ids = np.asarray(inputs['input_ids'], dtype=np.int32)
    labels = np.asarray(inputs['labels'], dtype=np.int32)
    shared = {
        'emb': f32('emb'),
        'wihT_f': np.ascontiguousarray(f32('Wih_f').T), 'wihT_b': np.ascontiguousarray(f32('Wih_b').T),
        'whhT_f': np.ascontiguousarray(f32('Whh_f').T), 'whhT_b': np.ascontiguousarray(f32('Whh_b').T),
        'bih_f': f32('bih_f'), 'bih_b': f32('bih_b'),
        'bhh_f': f32('bhh_f'), 'bhh_b': f32('bhh_b'),
        'wlinT': np.ascontiguousarray(f32('Wlin').T),
        'blin': f32('blin'), 'trans': f32('trans'), 'start': f32('start'), 'end': f32('end'),
    }
    in_maps = []
    for c in range(NCORES):
        bs = slice(c * B, (c + 1) * B)
        ids_flat = np.ascontiguousarray(input_ids[bs].T).reshape(-1)      # token = t*B+b
        ids_perm = np.ascontiguousarray(ids_flat.reshape(NTOK // 128, 128).T)  # [128, 8]
        lab_flat = np.ascontiguousarray(labels[bs].T).reshape(1, -1).astype(np.float32)
        in_maps.append({**shared, 'ids': ids_perm, 'lab': lab_flat})

    res = run_bass_kernel_spmd(nc, in_maps, core_ids=list(range(NCORES)),
                               trace=bool(os.environ.get('KERNEL_TRACE')))
    _cache['last_result'] = res
    dec = np.concatenate([res.results[c]['out_dec'] for c in range(NCORES)], axis=0).astype(np.int32)
    llh = np.concatenate([res.results[c]['out_stat'].reshape(-1)[:B] for c in range(NCORES)])
    loss = np.float32(-np.mean(llh))
    return dec, loss
